# revision 1
# baseline (speedup 1.0000x reference)
"""Trainium2 Bass kernel for nn_LookAtMappingNetwork (gnn_message_passing).

Strategy
--------
The module's output only reads the final node features at rows R = {i*250 :
i in 0..63} (``ws = x[::250]``).  Working backwards through the two message
-passing processors, only a small data-dependent subset of edges/nodes can
influence those rows, for ANY edge_index:

    E1 = edges with dst in R          (~6 per graph)   -> proc-1 edge MLP
    S  = R  ∪  src[E1]                (~65 per core)   -> rows where x1 needed
    E0 = edges with dst in S          (~375 per core)  -> proc-0 edge MLP

Segment-mean counts stay exact because E0/E1 contain ALL edges landing on
S/R.  Everything else the reference computes is dead code.  Each of the 8
cores handles 8 output rows (its R_c) fully independently.

Performance layout
------------------
* All FC weights are transposed, pre-scaled by (lr/sqrt(fan_in))*sqrt(2)
  and packed host-side into ONE bf16 tensor of 128-row K-tiles.  Since
  leaky_relu commutes with positive scaling, each layer's activation
  collapses to a single DVE op  out = max(psum, 0.2*psum)  with zero
  scalar-engine work on the critical path.
* Biases enter PSUM as K=1 matmul rows issued FIRST (start=True), so they
  are off the dependence tail; for the e00/n00 layers they ride spare
  partition rows of packed combo K-tiles.
* All host-marshalled metadata (z, %B selectors, slot ids, pre-gathered
  look_at rows) travels in two f32 tensors -> 2 DMA instructions; weights
  in 5 large DMAs split across both HWDGE rings in first-use order.  This
  removes ~45 small DMAs (600ns serial issue each) and 7 serialized
  indirect gathers from the old front-end.
* PSUM->SBUF copies alternate Vector/Scalar engines; transposes stay on
  the PE (bf16 identity matmuls).
* Output is written un-replicated [8, 512]; the x14 ws broadcast happens
  on the host.
"""

import math

import ml_dtypes
import numpy as np

import concourse.bacc as bacc
import concourse.bass as bass
import concourse.mybir as mybir
import concourse.tile as tile
from concourse.bass_utils import run_bass_kernel_spmd
from concourse.masks import make_identity

f32 = mybir.dt.float32
fr = mybir.dt.bfloat16
i32 = mybir.dt.int32
AF = mybir.ActivationFunctionType
OP = mybir.AluOpType

NV = 250
B = 64
D = 512
LR = 0.01
SQ2 = math.sqrt(2.0)
N_CORES = 8
R_PER = B // N_CORES  # output rows per core

CAP_E0 = 384
CAP_S = 128
CAP_E1 = 128
NT0 = CAP_E0 // 128

G_E00 = LR / math.sqrt(1034.0)
G_E01 = LR / math.sqrt(512.0)
G_N00 = LR / math.sqrt(1030.0)
G_N01 = LR / math.sqrt(512.0)
G_E10 = LR / math.sqrt(1536.0)
G_E11 = LR / math.sqrt(512.0)
G_N10 = LR / math.sqrt(1024.0)
G_N11 = LR / math.sqrt(512.0)

# ---- packed weight tile indices (each tile = [128, 512] bf16) ----
# The pack is stored PAIR-INTERLEAVED in DRAM (rows (q*128+p)*2+j for tile
# pair q, partition p, j in {0,1}) so each DMA partition line is a 2 KiB
# contiguous run -- double the per-descriptor efficiency of 1 KiB lines.
# PAD tiles sit at group boundaries: each weight-DMA group rewrites the
# previous group's pad (or a late-consumed) tile, giving a WAW hazard that
# makes the Tile scheduler chain the transfers in first-use order
# (otherwise the SDMA engines round-robin ALL queued DMAs and the first
# weights arrive last).  Group bounds are even so pairing stays aligned.
T_ZSRC = 0     # 4 tiles: w0e0^T rows 0:512    (z of src)
T_ZDST = 4     # 4 tiles: w0e0^T rows 515:1027 (z of dst)
T_BROWS_E = 8  # bias rows for e01@0 / n01@32 / e10@64
T_LARAW = 9    # 0:3 laA-rel | 32:35 laB+rel | 96:97 wd | 97:98 b_e00
T_BROWS_L = 10  # bias rows for e11@0 / n10@32 / n11@64 (late-consumed)
# pad 11                                   GA = [0:12]
T_W0E1 = 12    # 4
T_W0N0Z = 16   # 4: w0n0^T rows 0:512
T_W0N0A = 20   # 4: w0n0^T rows 515:1027 (agg features 0:512)
T_N00C = 24    # 1: 0:3 la | 32:35 agg tail | 64:65 b_n00
T_W0N1 = 25    # 4
# spare 29, pads 30,31                     GB = [10:32]
T_W1E1 = 32    # 4
T_W1N1 = 36    # 4
T_W1N0 = 40    # 8
NT = 48        #                           GC = [30:48]

# w1e0 (12 tiles) ships as fp8_e4m3 (scaled 2^6 to clear the subnormal
# range; the e10 lrelu rescales by 2^-6 for free) on the otherwise-idle
# scalar ring.  Its quantization noise is attenuated by the E1 edge-mean
# before reaching the output, unlike w1n0's, which stays bf16 (both in
# fp8 measured rel err 2.8e-2 -- over the 2e-2 gate).  The chained sync
# stream drops to ~6 MB and the fp8 pack is only 0.75 MB.
T8_W1E0 = 0    # 12 tiles (fp8 pack)
NT8 = 12
F8S = 64.0

# brow key -> (tile, partition base): matmul bases must be 0/32/64
BROW_SLOT = {"e01": (T_BROWS_E, 0), "n01": (T_BROWS_E, 32),
             "e10": (T_BROWS_E, 64), "e11": (T_BROWS_L, 0),
             "n10": (T_BROWS_L, 32), "n11": (T_BROWS_L, 64)}

# ---- meta128 [128, 409] f32 column layout ----
C_SIG = 0      # 3 cols: e0 sigma (dst slot in S) per e-tile
C_E1SIG = 3    # e1 sigma (dst slot in R)
C_E1POS = 4    # 128: e1 -> position in E0
C_E1SRC = 132  # 128: e1 src slot in S
C_E1DST = 260  # 128: e1 dst slot in S
C_LAS = 388    # 3: look_ats[S]
C_LASRC = 391  # 9: look_ats[e0 src], 3 cols per e-tile
C_LADST = 400  # 9: look_ats[e0 dst]
C_RIN0 = 409   # 1/max(count,1) per S slot (host-computed from indices)
C_RIN1 = 410   # 1/max(count,1) per R slot
M128F = 416    # padded for DMA row alignment

# ---- meta64 [64, 1408] f32 column layout ----
Z0 = 0         # 512: z
C_SMOD = 512   # 384: e0 src % B
C_DMOD = 896   # 384: e0 dst % B
C_SSEL = 1280  # 128: S % B
M64F = 1408

k4 = [(0, 128), (128, 256), (256, 384), (384, 512)]


def _build_program():
    nc = bacc.Bacc("TRN2", target_bir_lowering=False, debug=False,
                   enable_asserts=False, num_devices=N_CORES)

    wpack_d = nc.dram_tensor("wpack", [NT * 128, 512], fr, kind="ExternalInput")
    wpack8_d = nc.dram_tensor("wpack8", [NT8 * 128, 512], mybir.dt.float8e4,
                              kind="ExternalInput")
    m64_d = nc.dram_tensor("m64", [64, M64F], f32, kind="ExternalInput")
    m128_d = nc.dram_tensor("m128", [128, M128F], f32, kind="ExternalInput")
    out_d = nc.dram_tensor("out", [R_PER, D], f32, kind="ExternalOutput")

    with tile.TileContext(nc) as tc, \
            tc.tile_pool(name="w", bufs=1) as wp, \
            tc.tile_pool(name="tmp", bufs=8) as tp, \
            tc.tile_pool(name="psb", bufs=4, space="PSUM") as psb, \
            tc.tile_pool(name="pss", bufs=4, space="PSUM") as pss:

        # ---------------- input DMAs (two HWDGE rings, first-use order) ---
        m64 = wp.tile([64, M64F], f32, name="m64")
        nc.scalar.dma_start(m64[:], m64_d[:, :])
        m128 = wp.tile([128, M128F], f32, name="m128")
        nc.scalar.dma_start(m128[:], m128_d[:, :])

        wbig = wp.tile([128, NT, 512], fr, name="wbig")

        def wload(eng, a, b_):
            eng.dma_start(
                wbig[:, a:b_, :].rearrange("p (q j) d -> p q j d", j=2),
                wpack_d[128 * a:128 * b_, :].rearrange(
                    "(q p j) d -> p q j d", p=128, j=2))

        # One chained stream on the sync ring, in first-use order; each
        # group overlaps the previous group's pad (or late-consumed) tile,
        # the WAW hazard serializing the transfers so early weights land
        # early instead of all DMAs finishing together.
        wload(nc.sync, 0, 12)          # zsrc, zdst, brows, laraw
        wload(nc.sync, 10, 32)         # w0e1, w0n0, n00 combo, w0n1
        wload(nc.sync, 30, 48)         # w1e1, w1n1, w1n0

        # fp8 pack (quad-interleaved -> 2 KiB lines) on the scalar ring,
        # arriving well before first use; no chain needed.
        wbig8 = wp.tile([128, NT8, 512], mybir.dt.float8e4, name="wbig8")
        nc.scalar.dma_start(
            wbig8[:, :, :].rearrange("p (q j) d -> p q j d", j=4),
            wpack8_d[:, :].rearrange("(q p j) d -> p q j d", p=128, j=4))

        def W8(i):
            return wbig8[:, i, :]

        def W(i):
            return wbig[:, i, :]

        # ---------------- constants ----------------
        ident_f = wp.tile([128, 128], f32, name="ident_f")
        make_identity(nc, ident_f[:])
        ident = wp.tile([128, 128], fr, name="ident")
        nc.vector.tensor_copy(ident[:], ident_f[:])
        idents = {fr: ident, f32: ident_f}
        ones_f32 = wp.tile([128, 1], f32, name="ones_f32")
        nc.gpsimd.memset(ones_f32[:], 1.0)
        zeros_f32 = wp.tile([128, 1], f32, name="zeros_f32")
        nc.gpsimd.memset(zeros_f32[:], 0.0)
        iota_free = wp.tile([128, 128], f32, name="iota_free")
        nc.gpsimd.iota(iota_free[:], pattern=[[1, 128]], base=0,
                       channel_multiplier=0, allow_small_or_imprecise_dtypes=True)
        iota_part = []
        for t in range(NT0):
            it = wp.tile([128, 1], f32, name=f"iota_part{t}")
            nc.gpsimd.iota(it[:], pattern=[[1, 1]], base=128 * t,
                           channel_multiplier=1,
                           allow_small_or_imprecise_dtypes=True)
            iota_part.append(it)
        # ones rows at partition bases 0/32/64 (for bias-row matmuls)
        ones_rows = wp.tile([65, 128], fr, name="ones_rows")
        nc.vector.tensor_copy(ones_rows[:], ones_f32[:65, :1].to_broadcast([65, 128]))

        _uid = [0]

        def uid():
            _uid[0] += 1
            return _uid[0]

        def sb(shape, name):
            return wp.tile(shape, fr, name=name)

        _cp = [0]

        def ps_copy(dst_ap, src_ap):
            """PSUM->SBUF copy, alternating Vector/Scalar engines."""
            _cp[0] += 1
            if _cp[0] % 2 == 0:
                nc.vector.tensor_copy(dst_ap, src_ap)
            else:
                nc.scalar.copy(dst_ap, src_ap)

        def copyT(src_ap, p, f, dst_ap):
            """PE transpose src [p, f] -> existing sbuf dst_ap [f, p]."""
            sdt = src_ap.dtype
            ps = pss.tile([f, p], sdt, name=f"psT{uid()}", tag="pssm")
            nc.tensor.transpose(ps[:], src_ap, idents[sdt][:p, :p])
            ps_copy(dst_ap, ps[:])

        def peT(src_ap, p, f, name):
            dst = sb([f, p], name)
            copyT(src_ap, p, f, dst[:])
            return dst

        def brow_mm(ps_t, key, p):
            tidx, pbase = BROW_SLOT[key]
            nc.tensor.matmul(ps_t[:], ones_rows[pbase:pbase + 1, :p],
                             wbig[pbase:pbase + 1, tidx, :],
                             start=True, stop=False)

        def lrelu(ps_ap, out_ap, s_copy=False):
            """out = leaky_relu(psum, 0.2) -- gain pre-folded into weights.
            (The DVE cannot read two PSUM operands, so stage through SBUF.)
            s_copy routes the staging copy to the Scalar engine so back-to-
            back tiles pipeline copy/stt across engines instead of
            serializing on the DVE (worth it for 128-partition tiles; for
            8-partition tiles the DVE copy is lower-latency)."""
            p, n = ps_ap.shape
            t = tp.tile([p, n], f32, name=f"lr{uid()}", tag=f"lr{p}_{n}")
            if s_copy:
                nc.scalar.copy(t[:], ps_ap)
            else:
                nc.vector.tensor_copy(t[:], ps_ap)
            nc.vector.scalar_tensor_tensor(out_ap, t[:], 0.2, ps_ap,
                                           op0=OP.mult, op1=OP.max)

        def lrelu_chunk(ps_t, out_t, p, consume, scale=None):
            """Chunked lrelu over 4 x 128 output columns; consume(c, out_ap)
            emits the chunk's consumers right away so the PE restarts while
            later chunks are still on the DVE.  First chunk's copy runs on
            Vector (lowest latency), the rest on Scalar in parallel.  With
            scale, the PSUM is rescaled during the staging copy (free) and
            the max reads the staged copy twice."""
            for c in range(4):
                cs = slice(128 * c, 128 * (c + 1))
                t = tp.tile([p, 128], f32, name=f"lrc{uid()}", tag=f"lrc{p}")
                if scale is None:
                    if c == 0:
                        nc.vector.tensor_copy(t[:], ps_t[:, cs])
                    else:
                        nc.scalar.copy(t[:], ps_t[:, cs])
                    nc.vector.scalar_tensor_tensor(out_t[:, cs], t[:], 0.2,
                                                   ps_t[:, cs],
                                                   op0=OP.mult, op1=OP.max)
                else:
                    if c == 0:
                        nc.vector.tensor_scalar_mul(t[:], ps_t[:, cs], scale)
                    else:
                        nc.scalar.activation(t[:], ps_t[:, cs], AF.Identity,
                                             bias=0.0, scale=scale)
                    nc.vector.scalar_tensor_tensor(out_t[:, cs], t[:], 0.2,
                                                   t[:],
                                                   op0=OP.mult, op1=OP.max)
                consume(c, out_t[:, cs])

        # rhs combo tile for the n00 layer (zero-filled now, rows set later)
        rhs_n00 = sb([128, CAP_S], "rhs_n00")
        nc.vector.tensor_copy(rhs_n00[:],
                              zeros_f32[:, :1].to_broadcast([128, CAP_S]))
        nc.vector.tensor_copy(rhs_n00[64:65, :],
                              ones_f32[:1, :1].to_broadcast([1, CAP_S]))

        # ---------------- z normalization ----------------
        zt = m64[:, Z0:Z0 + 512]
        zsq = tp.tile([64, 512], f32, name="zsq", tag="scr")
        zss = wp.tile([64, 1], f32, name="zss")
        nc.vector.tensor_tensor(zsq[:], zt, zt, op=OP.mult)
        nc.vector.tensor_reduce(zss[:], zsq[:], axis=mybir.AxisListType.X,
                                op=OP.add)
        nc.vector.tensor_scalar(zss[:], zss[:], 1.0 / 512.0, 1e-8,
                                OP.mult, OP.add)
        zsr = wp.tile([64, 1], f32, name="zsr")
        nc.scalar.sqrt(zsr[:], zss[:])
        zrin = wp.tile([64, 1], f32, name="zrin")
        nc.vector.reciprocal(zrin[:], zsr[:])
        znt = sb([64, 512], "znt")
        nc.vector.tensor_scalar_mul(znt[:], zt, zrin[:, :1])

        znT = []
        for k in range(4):
            znT.append(peT(znt[:64, 128 * k:128 * (k + 1)], 64, 128, f"znT{k}"))

        # ---------------- selector matrices (DVE, meta-derived) -----------
        def iseq(out_ap, in_ap, iota_t):
            nc.vector.tensor_scalar(out_ap, in_ap, iota_t, None, OP.is_equal)

        # ---------------- edge geometry: dist + laRhs ---------------------
        ds3 = tp.tile([128, 3], f32, name="ds3", tag="rel")
        for t in range(NT0):
            lasrc = m128[:, C_LASRC + 3 * t:C_LASRC + 3 * (t + 1)]
            ladst = m128[:, C_LADST + 3 * t:C_LADST + 3 * (t + 1)]
            rel = tp.tile([128, 3], f32, name=f"rel{t}", tag="rel")
            nc.vector.tensor_tensor(rel[:], ladst, lasrc, op=OP.subtract)
            sq = tp.tile([128, 3], f32, name=f"sq{t}", tag="rel")
            nc.vector.tensor_tensor(sq[:], rel[:], rel[:], op=OP.mult)
            nc.vector.tensor_reduce(ds3[:, t:t + 1], sq[:],
                                    axis=mybir.AxisListType.X, op=OP.add)
        dist3 = wp.tile([128, 3], f32, name="dist3")
        nc.scalar.sqrt(dist3[:], ds3[:])

        # laRhs: feature-major rhs [97 used rows, E0] matching laWc layout
        laRhs = sb([128, CAP_E0], "laRhs")
        for t in range(NT0):
            cmb = tp.tile([128, 128], f32, name=f"cmb{t}", tag="cmb")
            nc.vector.tensor_copy(cmb[:],
                                  zeros_f32[:, :1].to_broadcast([128, 128]))
            nc.vector.tensor_copy(cmb[:, 0:3],
                                  m128[:, C_LASRC + 3 * t:C_LASRC + 3 * (t + 1)])
            nc.vector.tensor_copy(cmb[:, 32:35],
                                  m128[:, C_LADST + 3 * t:C_LADST + 3 * (t + 1)])
            nc.vector.tensor_copy(cmb[:, 96:97], dist3[:, t:t + 1])
            nc.vector.tensor_copy(cmb[:, 97:98], ones_f32[:, :1])
            copyT(cmb[:], 128, 128, laRhs[:, 128 * t:128 * (t + 1)])

        # ---------------- zterm + zgS (PE) --------------------------------
        def zterm(base, name):
            ps = psb.tile([64, 512], f32, name=f"ps_{name}", tag="psbig")
            for k in range(4):
                nc.tensor.matmul(ps[:], znT[k][:], W(base + k),
                                 start=(k == 0), stop=(k == 3))
            t_ = sb([64, 512], name)
            ps_copy(t_[:], ps[:])
            return t_

        ztermA = zterm(T_ZSRC, "ztermA")
        ztermB = zterm(T_ZDST, "ztermB")

        selS = sb([64, CAP_S], "selS")
        iseq(selS[:], m64[:, C_SSEL:C_SSEL + CAP_S], iota_part[0][:64, :1])
        zgS = []
        for c in range(4):
            ps = pss.tile([128, CAP_S], f32, name=f"ps_zg{c}", tag="pssm")
            nc.tensor.matmul(ps[:], znt[:64, 128 * c:128 * (c + 1)], selS[:],
                             start=True, stop=True)
            t_ = sb([128, CAP_S], f"zgS{c}")
            ps_copy(t_[:], ps[:])
            zgS.append(t_)

        # ---------------- proc-0 edge MLP layer 1 (feature-major) ---------
        sel0s = sb([64, CAP_E0], "sel0s")
        iseq(sel0s[:], m64[:, C_SMOD:C_SMOD + CAP_E0], iota_part[0][:64, :1])
        sel0d = sb([64, CAP_E0], "sel0d")
        iseq(sel0d[:], m64[:, C_DMOD:C_DMOD + CAP_E0], iota_part[0][:64, :1])
        h0 = []
        for c in range(4):
            cs = slice(128 * c, 128 * (c + 1))
            ps = psb.tile([128, CAP_E0], f32, name=f"ps_efp{c}", tag="psbig")
            nc.tensor.matmul(ps[:], wbig[0:98, T_LARAW, cs], laRhs[0:98, :],
                             start=True, stop=False)
            nc.tensor.matmul(ps[:], ztermA[:64, cs], sel0s[:],
                             start=False, stop=False)
            nc.tensor.matmul(ps[:], ztermB[:64, cs], sel0d[:],
                             start=False, stop=True)
            o = sb([128, CAP_E0], f"h0_{c}")
            lrelu(ps[:], o[:], s_copy=True)
            h0.append(o)

        # ---------------- proc-0 edge MLP layer 2 (token-major) -----------
        msg = []
        for t in range(NT0):
            m = sb([128, 515], f"msg{t}")
            nc.vector.tensor_copy(m[:, 0:3],
                                  m128[:, C_LADST + 3 * t:C_LADST + 3 * (t + 1)])
            es = slice(128 * t, 128 * (t + 1))
            ps = psb.tile([128, 512], f32, name=f"ps_ef0{t}", tag="psbig")
            brow_mm(ps, "e01", 128)
            for k in range(4):
                nc.tensor.matmul(ps[:], h0[k][:, es], W(T_W0E1 + k),
                                 start=False, stop=(k == 3))
            lrelu(ps[:], m[:, 3:515], s_copy=True)
            msg.append(m)

        # ---------------- aggregation onto S ------------------------------
        G0 = []
        for t in range(NT0):
            g = sb([128, CAP_S], f"G0_{t}")
            iseq(g[:], iota_free[:, 0:CAP_S], m128[:, C_SIG + t:C_SIG + t + 1])
            G0.append(g)
        ps_a = psb.tile([CAP_S, 512], f32, name="ps_agg0a", tag="psbig")
        ps_b = pss.tile([CAP_S, 3], f32, name="ps_agg0b", tag="pssm")
        for t in range(NT0):
            nc.tensor.matmul(ps_a[:], G0[t][:], msg[t][:, 0:512],
                             start=(t == 0), stop=(t == NT0 - 1))
            nc.tensor.matmul(ps_b[:], G0[t][:], msg[t][:, 512:515],
                             start=(t == 0), stop=(t == NT0 - 1))
        rin = m128[0:CAP_S, C_RIN0:C_RIN0 + 1]
        aggtok = sb([CAP_S, 515], "aggtok")
        nc.vector.tensor_scalar_mul(aggtok[:, 0:512], ps_a[:, 0:512], rin)
        nc.vector.tensor_scalar_mul(aggtok[:, 512:515], ps_b[:, 0:3], rin)
        aggT = []
        for c in range(4):
            aggT.append(peT(aggtok[:, 128 * c:128 * (c + 1)], CAP_S, 128,
                            f"aggT{c}"))
        # small rows of the n00 rhs combo: la(S) and the agg tail
        copyT(m128[0:CAP_S, C_LAS:C_LAS + 3], CAP_S, 3, rhs_n00[0:3, :])
        copyT(aggtok[:, 512:515], CAP_S, 3, rhs_n00[32:35, :])

        # ---------------- node MLP 0 -> x1 (token-major, S slots) ---------
        ps = psb.tile([CAP_S, 512], f32, name="ps_n00", tag="psbig")
        for c in range(4):
            nc.tensor.matmul(ps[:], zgS[c][:], W(T_W0N0Z + c),
                             start=(c == 0), stop=False)
        for c in range(4):
            nc.tensor.matmul(ps[:], aggT[c][:], W(T_W0N0A + c),
                             start=False, stop=False)
        nc.tensor.matmul(ps[:], rhs_n00[0:97, :], wbig[0:97, T_N00C, :],
                         start=False, stop=True)
        hn_tok = sb([CAP_S, 512], "hn_tok")
        hnT = [sb([128, CAP_S], f"hnT{c}") for c in range(4)]
        lrelu_chunk(ps[:], hn_tok[:], CAP_S,
                    lambda c, ap: copyT(ap, CAP_S, 128, hnT[c][:]))

        ps = psb.tile([CAP_S, 512], f32, name="ps_n01", tag="psbig")
        brow_mm(ps, "n01", CAP_S)
        for c in range(4):
            nc.tensor.matmul(ps[:], hnT[c][:], W(T_W0N1 + c),
                             start=False, stop=(c == 3))
        x1tok = sb([CAP_S, 512], "x1tok")
        x1R = [sb([128, R_PER], f"x1R{c}") for c in range(4)]

        def x1_consume(c, ap):
            ps_ = pss.tile([128, R_PER], f32, name=f"ps_x1R{c}", tag="pssm")
            nc.tensor.matmul(ps_[:], ap, ident[:CAP_S, 0:R_PER],
                             start=True, stop=True)
            ps_copy(x1R[c][:], ps_[:])

        lrelu_chunk(ps[:], x1tok[:], CAP_S, x1_consume)

        # ---------------- proc-1 edge gathers (selection matmuls) ---------
        def sel_gather(lhsT_fns, sel_tiles, name):
            outs = []
            for c in range(4):
                ps_ = pss.tile([128, CAP_E1], f32, name=f"ps_{name}{c}",
                               tag="pssm")
                for t, s_ in enumerate(sel_tiles):
                    nc.tensor.matmul(ps_[:], lhsT_fns[t](c), s_[:],
                                     start=(t == 0),
                                     stop=(t == len(sel_tiles) - 1))
                o = sb([128, CAP_E1], f"{name}{c}")
                ps_copy(o[:], ps_[:])
                outs.append(o)
            return outs

        selA = sb([CAP_S, CAP_E1], "selA")
        iseq(selA[:], m128[0:CAP_S, C_E1SRC:C_E1SRC + CAP_E1],
             iota_part[0][:CAP_S, :1])
        selB = sb([CAP_S, CAP_E1], "selB")
        iseq(selB[:], m128[0:CAP_S, C_E1DST:C_E1DST + CAP_E1],
             iota_part[0][:CAP_S, :1])
        selE = []
        for t in range(NT0):
            s_ = sb([128, CAP_E1], f"selE{t}")
            iseq(s_[:], m128[:, C_E1POS:C_E1POS + CAP_E1], iota_part[t][:, :1])
            selE.append(s_)
        ef0g = sel_gather(
            [(lambda t: (lambda c: msg[t][:, 3 + 128 * c:3 + 128 * (c + 1)]))(t)
             for t in range(NT0)], selE, "ef0g")
        x1gA = sel_gather([lambda c: x1tok[:, 128 * c:128 * (c + 1)]], [selA],
                          "x1gA")
        x1gB = sel_gather([lambda c: x1tok[:, 128 * c:128 * (c + 1)]], [selB],
                          "x1gB")

        # ---------------- proc-1 edge MLP (token-major, E1) ---------------
        ps = psb.tile([CAP_E1, 512], f32, name="ps_e10", tag="psbig")
        brow_mm(ps, "e10", CAP_E1)
        for i, grp in enumerate(ef0g + x1gA + x1gB):
            widx = [8, 9, 10, 11, 0, 1, 2, 3, 4, 5, 6, 7][i]
            nc.tensor.matmul(ps[:], grp[:], W8(T8_W1E0 + widx),
                             start=False, stop=(i == 11))
        h1tok = sb([CAP_E1, 512], "h1tok")
        h1T = [sb([128, CAP_E1], f"h1T{c}") for c in range(4)]
        lrelu_chunk(ps[:], h1tok[:], CAP_E1,
                    lambda c, ap: copyT(ap, CAP_E1, 128, h1T[c][:]),
                    scale=1.0 / F8S)
        # e11 chunks feed the R-aggregation (matmul + scale + transpose)
        # as soon as each 128-col slice of ef1 is ready.
        msg1 = sb([CAP_E1, 512], "msg1")
        G1 = sb([CAP_E1, R_PER], "G1")
        iseq(G1[:], iota_free[:CAP_E1, 0:R_PER],
             m128[0:CAP_E1, C_E1SIG:C_E1SIG + 1])
        ps = psb.tile([CAP_E1, 512], f32, name="ps_e11", tag="psbig")
        brow_mm(ps, "e11", CAP_E1)
        for c in range(4):
            nc.tensor.matmul(ps[:], h1T[c][:], W(T_W1E1 + c),
                             start=False, stop=(c == 3))
        rin1 = m128[0:R_PER, C_RIN1:C_RIN1 + 1]
        ps1 = psb.tile([R_PER, 512], f32, name="ps_agg1", tag="psbig")
        agg1tok = sb([R_PER, 512], "agg1tok")
        agg1T = [sb([128, R_PER], f"agg1T{c}") for c in range(4)]

        def e11_consume(c, ap):
            cs = slice(128 * c, 128 * (c + 1))
            nc.tensor.matmul(ps1[:, cs], G1[:], ap, start=True, stop=True)
            nc.vector.tensor_scalar_mul(agg1tok[:, cs], ps1[:, cs], rin1)
            copyT(agg1tok[:R_PER, cs], R_PER, 128, agg1T[c][:])

        lrelu_chunk(ps[:], msg1[:], CAP_E1, e11_consume)

        # ---------------- final node MLP (token-major, 8 rows) ------------
        ps = psb.tile([R_PER, 512], f32, name="ps_n10", tag="psbig")
        brow_mm(ps, "n10", R_PER)
        for i, grp in enumerate(x1R + agg1T):
            nc.tensor.matmul(ps[:], grp[:], W(T_W1N0 + i),
                             start=False, stop=(i == 7))
        hftok = sb([R_PER, 512], "hftok")
        lrelu(ps[:], hftok[:])
        hfT = []
        for c in range(4):
            hfT.append(peT(hftok[:R_PER, 128 * c:128 * (c + 1)], R_PER, 128,
                           f"hfT{c}"))
        ps = psb.tile([R_PER, 512], f32, name="ps_n11", tag="psbig")
        brow_mm(ps, "n11", R_PER)
        for c in range(4):
            nc.tensor.matmul(ps[:], hfT[c][:], W(T_W1N1 + c),
                             start=False, stop=(c == 3))
        wstok = wp.tile([R_PER, 512], f32, name="wstok")
        lrelu(ps[:], wstok[:])

        nc.sync.dma_start(out_d[:, :], wstok[:, :])


    nc.finalize()
    return nc


_PROG_CACHE = {}


def _get_program():
    key = (CAP_E0, CAP_S, CAP_E1)
    if key not in _PROG_CACHE:
        _PROG_CACHE[key] = _build_program()
    return _PROG_CACHE[key]


def _pad(a, n, fill):
    out = np.full((n,), fill, dtype=np.float32)
    out[:len(a)] = a.astype(np.float32)
    return out


def _host_weights(inputs):
    """Pack all FC weights (transposed, gain*sqrt2 pre-folded) + biases
    into one [NT*128, 512] bf16 tensor of K-tiles."""
    f = np.float32
    s = SQ2

    def T(name):
        return np.ascontiguousarray(np.asarray(inputs[name], f).T)

    w0e0T, w0e1T = T("p0_ew0"), T("p0_ew1")
    w0n0T, w0n1T = T("p0_nw0"), T("p0_nw1")
    w1e0T, w1e1T = T("p1_ew0"), T("p1_ew1")
    w1n0T, w1n1T = T("p1_nw0"), T("p1_nw1")

    def bias(name):
        return np.asarray(inputs[name], f)

    wpk = np.zeros((NT * 128, 512), f)

    def put(idx, rows):
        wpk[idx * 128: idx * 128 + rows.shape[0]] = rows

    put(T_ZSRC, w0e0T[0:512] * (G_E00 * s))
    put(T_ZDST, w0e0T[515:1027] * (G_E00 * s))
    for key, bname in [("e01", "p0_eb1"), ("n01", "p0_nb1"),
                       ("e10", "p1_eb0"), ("e11", "p1_eb1"),
                       ("n10", "p1_nb0"), ("n11", "p1_nb1")]:
        tidx, pbase = BROW_SLOT[key]
        bsc = F8S if key == "e10" else 1.0
        wpk[tidx * 128 + pbase] = bias(bname) * (LR * s * bsc)
    # rel = la[dst]-la[src] folds into the src/dst la blocks:
    #   src rows get (laA - w_rel), dst rows get (laB + w_rel)
    laraw = np.zeros((128, 512), f)
    laraw[0:3] = (w0e0T[512:515] - w0e0T[1030:1033]) * (G_E00 * s)
    laraw[32:35] = (w0e0T[1027:1030] + w0e0T[1030:1033]) * (G_E00 * s)
    laraw[96:97] = w0e0T[1033:1034] * (G_E00 * s)  # dist weight
    laraw[97] = bias("p0_eb0") * (LR * s)
    put(T_LARAW, laraw)
    put(T_W0E1, w0e1T * (G_E01 * s))  # noqa: placeholder-anchor
    put(T_W0N0Z, w0n0T[0:512] * (G_N00 * s))
    put(T_W0N0A, w0n0T[515:1027] * (G_N00 * s))
    comb = np.zeros((128, 512), f)
    comb[0:3] = w0n0T[512:515] * (G_N00 * s)      # la features of x
    comb[32:35] = w0n0T[1027:1030] * (G_N00 * s)  # agg tail (512:515)
    comb[64] = bias("p0_nb0") * (LR * s)
    put(T_N00C, comb)
    put(T_W0N1, w0n1T * (G_N01 * s))
    put(T_W1E1, w1e1T * (G_E11 * s))
    put(T_W1N1, w1n1T * (G_N11 * s))
    put(T_W1N0, w1n0T * (G_N10 * s))
    wpk8 = np.zeros((NT8 * 128, 512), f)
    wpk8[T8_W1E0 * 128:(T8_W1E0 + 12) * 128] = w1e0T * (G_E10 * s * F8S)
    wpk8 = wpk8.reshape(NT8 // 4, 4, 128, 512).transpose(0, 2, 1, 3)
    wpk8 = np.ascontiguousarray(wpk8.reshape(NT8 * 128, 512))
    wpk8 = np.ascontiguousarray(wpk8.astype(ml_dtypes.float8_e4m3))
    # pair-interleave rows: tile pair q -> rows (q*128+p)*2+j
    wpk = wpk.reshape(NT // 2, 2, 128, 512).transpose(0, 2, 1, 3)
    wpk = np.ascontiguousarray(wpk.reshape(NT * 128, 512))
    return np.ascontiguousarray(wpk.astype(ml_dtypes.bfloat16)), wpk8


def _core_meta(z, la, src, dst, c):
    """Per-core metadata tensors (integer index-set construction + row
    gathers of input data; no arithmetic on tensor values)."""
    Rc = (np.arange(R_PER, dtype=np.int64) + c * R_PER) * NV
    E1 = np.nonzero(np.isin(dst, Rc))[0]
    others = np.setdiff1d(np.unique(src[E1]), Rc)
    S = np.concatenate([Rc, others])
    assert len(E1) <= CAP_E1 and len(S) <= CAP_S, (len(E1), len(S))
    slot = np.full(16000, -1, np.int64)
    slot[S] = np.arange(len(S))
    E0 = np.nonzero(slot[dst] >= 0)[0]
    assert len(E0) <= CAP_E0, len(E0)
    pos = np.full(src.shape[0], -1, np.int64)
    pos[E0] = np.arange(len(E0))
    e0s, e0d = src[E0], dst[E0]
    e1s, e1d = src[E1], dst[E1]

    def gat(idx, n):
        out = np.zeros((n, 3), np.float32)
        out[:len(idx)] = la[idx]
        return out

    m128 = np.zeros((128, M128F), np.float32)
    m128[:, C_SIG:C_SIG + NT0] = _pad(slot[e0d], CAP_E0, -1).reshape(NT0, 128).T
    m128[0:CAP_E1, C_E1SIG] = _pad(slot[e1d], CAP_E1, -1)
    m128[:, C_E1POS:C_E1POS + CAP_E1] = _pad(pos[E1], CAP_E1, -1)[None, :]
    m128[:, C_E1SRC:C_E1SRC + CAP_E1] = _pad(slot[e1s], CAP_E1, -1)[None, :]
    m128[:, C_E1DST:C_E1DST + CAP_E1] = _pad(slot[e1d], CAP_E1, -1)[None, :]
    m128[0:CAP_S, C_LAS:C_LAS + 3] = gat(S, CAP_S)
    cnt0 = np.bincount(slot[e0d].astype(np.int64), minlength=CAP_S)[:CAP_S]
    m128[0:CAP_S, C_RIN0] = 1.0 / np.maximum(cnt0, 1)
    cnt1 = np.bincount(slot[e1d].astype(np.int64), minlength=R_PER)[:R_PER]
    m128[0:R_PER, C_RIN1] = 1.0 / np.maximum(cnt1, 1)
    la_s = gat(e0s, CAP_E0).reshape(NT0, 128, 3)
    la_d = gat(e0d, CAP_E0).reshape(NT0, 128, 3)
    for t in range(NT0):
        m128[:, C_LASRC + 3 * t:C_LASRC + 3 * (t + 1)] = la_s[t]
        m128[:, C_LADST + 3 * t:C_LADST + 3 * (t + 1)] = la_d[t]

    m64 = np.zeros((64, M64F), np.float32)
    m64[:, Z0:Z0 + 512] = z
    m64[:, C_SMOD:C_SMOD + CAP_E0] = _pad(e0s % B, CAP_E0, 0)[None, :]
    m64[:, C_DMOD:C_DMOD + CAP_E0] = _pad(e0d % B, CAP_E0, 0)[None, :]
    m64[:, C_SSEL:C_SSEL + CAP_S] = _pad(S % B, CAP_S, 0)[None, :]
    return {"m64": m64, "m128": np.ascontiguousarray(m128)}


def make_in_maps(inputs):
    ei = np.asarray(inputs["edge_index"])
    src, dst = ei[0].astype(np.int64), ei[1].astype(np.int64)
    z = np.ascontiguousarray(np.asarray(inputs["z"], np.float32))
    la = np.ascontiguousarray(np.asarray(inputs["look_ats"], np.float32))
    wpk, wpk8 = _host_weights(inputs)
    return [dict(wpack=wpk, wpack8=wpk8, **_core_meta(z, la, src, dst, c))
            for c in range(N_CORES)]


def kernel(**inputs):
    nc = _get_program()
    in_maps = make_in_maps(inputs)
    res = run_bass_kernel_spmd(nc, in_maps, core_ids=list(range(N_CORES)))
    ws = np.concatenate([res.results[c]["out"] for c in range(N_CORES)],
                        axis=0).astype(np.float32)
    return np.ascontiguousarray(
        np.broadcast_to(ws[:, None, :], (B, 14, D))).astype(np.float32)

